# revision 29
# baseline (speedup 1.0000x reference)
"""Trainium2 Bass kernel for LiquidNeuronLayer.

Math (per batch b, hidden vec h in R^H):
    xw  = x @ Wx.T + b                      (B, T, H)   precomputed, phase 1
    tau = softplus(x @ tau_w.T + tau_b)+.01 (B, T, H)   precomputed, phase 1
    a   = DT / tau                          (B, T, H)   precomputed, phase 1
    per t:  z = tanh(xw_t + h @ Wh.T);  h = h + a_t * (z - h)    phase 2

Distribution: data-parallel over batch, B=16 over 8 cores -> Bc=2 per core.
Device layout is "(H, B)": hidden index h = jc*128 + p lives on partition p,
chunk jc in the free dim.  The recurrence matmul keeps Wh^T tiles stationary
(lhsT = WhT[kc,jc] 128x128) and streams the tiny h^T (128, Bc) as the moving
operand, so all elementwise work runs on [128, 8] tiles (full partition use).

Host does layout-only transforms (transposes/reshapes); all FLOPs on device.
"""

import numpy as np

import concourse.bass as bass
import concourse.mybir as mybir
import concourse.tile as tile
from concourse import bacc
from concourse.bass import ds
from concourse.bass_utils import run_bass_kernel_spmd

DT = 0.1
N_CORES = 8
P = 128

AFT = mybir.ActivationFunctionType
F32 = mybir.dt.float32
R32G = mybir.dt.float32r


def build_program(Bc=2, T=2048, D=256, H=512, TB=128, TBLK=512, mm_dt=None):
    """Build the per-core Bass program (same NEFF on all 8 cores).

    mm_dt: dtype of the recurrence matmul operands (Wh tiles and the cast of
    h fed to the PE).  fp32 runs the PE in two-pass LOW_HIGH mode (2x
    LDWEIGHTS + 2x MATMUL per tile); float16 runs single-pass with fast
    weight load.  h state / PSUM accumulation stay fp32 either way.
    """
    if mm_dt is None:
        mm_dt = _mm_cfg()[0]
    # "fp16w": fp16 stationary weights (fast weight load) with the h state
    # kept/streamed as float32r -- no per-step cast op.
    fp16w = mm_dt == "fp16w"
    if fp16w:
        mm_dt = mybir.dt.float16
    KC = H // P   # h-chunks (contraction side)
    JC = H // P   # h-chunks (output side)
    DC = D // P   # d-chunks
    n_tblk = T // TBLK

    nc = bacc.Bacc("TRN2", target_bir_lowering=False, debug=False)

    # ---- I/O ----
    xT_in = nc.dram_tensor("xT", [P, DC, Bc, T], R32G, kind="ExternalInput")
    whT_in = nc.dram_tensor("WhT", [P, KC, JC, P], mm_dt, kind="ExternalInput")
    wxT_in = nc.dram_tensor("WxT", [P, DC, JC, P], R32G, kind="ExternalInput")
    twT_in = nc.dram_tensor("TwT", [P, DC, JC, P], R32G, kind="ExternalInput")
    b_in = nc.dram_tensor("bv", [P, JC], F32, kind="ExternalInput")
    tb_in = nc.dram_tensor("tbv", [P, JC], F32, kind="ExternalInput")
    eye_in = nc.dram_tensor("eye", [P, P], mm_dt, kind="ExternalInput")
    R32 = mybir.dt.float32r
    state_dt = R32 if (mm_dt == R32 or fp16w) else F32
    # out_s[jc, b, p, t] = h_{b,t}[jc*128+p]; host re-layouts to (B,T,H).
    out_s = nc.dram_tensor("out_s", [JC, Bc, P, T], state_dt, kind="ExternalOutput")

    with tile.TileContext(nc) as tc:
        with (
            tc.tile_pool(name="const", bufs=1) as constp,
            tc.tile_pool(name="dram", bufs=1, space=bass.MemorySpace.DRAM) as dramp,
        ):
            whsb = constp.tile([P, KC, JC, P], mm_dt, tag="whsb")
            wxsb = constp.tile([P, DC, JC, P], R32G, tag="wxsb")
            twsb = constp.tile([P, DC, JC, P], R32G, tag="twsb")
            bsb = constp.tile([P, JC], F32, tag="bsb")
            tbsb = constp.tile([P, JC], F32, tag="tbsb")
            eyesb = constp.tile([P, P], mm_dt, tag="eyesb")
            nc.sync.dma_start(eyesb[:], eye_in[:])
            nc.sync.dma_start(whsb[:], whT_in[:])
            nc.sync.dma_start(wxsb[:], wxT_in[:])
            nc.sync.dma_start(twsb[:], twT_in[:])
            nc.sync.dma_start(bsb[:], b_in[:])
            nc.sync.dma_start(tbsb[:], tb_in[:])

            # DRAM scratch for the precomputed streams, [jc, b, p, t].
            xw_s = dramp.tile([JC, Bc, P, T], mm_dt, tag="xw_s")
            a_s = dramp.tile([JC, Bc, P, T], F32, tag="a_s")
            c_s = dramp.tile([JC, Bc, P, T], F32, tag="c_s")

            # ---------------- Phase 1: xw and a = DT/tau ----------------
            with (
                tc.tile_pool(name="p1x", bufs=2) as p1x,
                tc.tile_pool(name="p1o", bufs=3) as p1o,
                tc.tile_pool(name="p1ps", bufs=2, space=bass.MemorySpace.PSUM) as p1ps,
            ):
                for b_l in range(Bc):
                    for tb in range(n_tblk):
                        t0 = tb * TBLK
                        xts = p1x.tile([P, DC, TBLK], R32G, tag="xts")
                        for dc in range(DC):
                            nc.sync.dma_start(
                                xts[:, dc, :], xT_in[:, dc, b_l, ds(t0, TBLK)]
                            )
                        for jc in range(JC):
                            ps_xw = p1ps.tile([P, TBLK], F32, tag="ps_xw")
                            for dc in range(DC):
                                nc.tensor.matmul(
                                    ps_xw[:],
                                    wxsb[:, dc, jc, :],
                                    xts[:, dc, :],
                                    start=(dc == 0),
                                    stop=(dc == DC - 1),
                                )
                            xw_sb = p1o.tile([P, TBLK], mm_dt, tag="xw_sb")
                            nc.scalar.activation(
                                xw_sb[:], ps_xw[:], AFT.Identity,
                                bias=bsb[:, jc : jc + 1], scale=1.0,
                            )
                            nc.sync.dma_start(
                                xw_s[jc, b_l, :, ds(t0, TBLK)], xw_sb[:]
                            )

                            ps_ta = p1ps.tile([P, TBLK], F32, tag="ps_ta")
                            for dc in range(DC):
                                nc.tensor.matmul(
                                    ps_ta[:],
                                    twsb[:, dc, jc, :],
                                    xts[:, dc, :],
                                    start=(dc == 0),
                                    stop=(dc == DC - 1),
                                )
                            # softplus(u) = ln(exp(u) + 1); u = psum + tau_b
                            # stays in [-8, 8] here so exp cannot overflow.
                            eu_sb = p1o.tile([P, TBLK], F32, tag="eu_sb")
                            nc.scalar.activation(
                                eu_sb[:], ps_ta[:], AFT.Exp,
                                bias=tbsb[:, jc : jc + 1], scale=1.0,
                            )
                            sp_sb = p1o.tile([P, TBLK], F32, tag="sp_sb")
                            nc.scalar.activation(
                                sp_sb[:], eu_sb[:], AFT.Ln, bias=1.0, scale=1.0,
                            )
                            nc.vector.tensor_scalar_add(sp_sb[:], sp_sb[:], 0.01)
                            a_sb = p1o.tile([P, TBLK], F32, tag="a_sb")
                            nc.vector.reciprocal_approx_fast(a_sb[:], sp_sb[:])
                            nc.vector.tensor_scalar_mul(a_sb[:], a_sb[:], DT)
                            nc.sync.dma_start(
                                a_s[jc, b_l, :, ds(t0, TBLK)], a_sb[:]
                            )
                            c_sb = p1o.tile([P, TBLK], F32, tag="c_sb")
                            nc.vector.tensor_scalar(
                                c_sb[:], a_sb[:], -1.0, 1.0,
                                mybir.AluOpType.mult, mybir.AluOpType.add,
                            )
                            nc.sync.dma_start(
                                c_s[jc, b_l, :, ds(t0, TBLK)], c_sb[:]
                            )

            # ---------------- Phase 2: the recurrence ----------------
            # mm-state dtype: what the Wh matmuls consume
            hm_dt = mm_dt if mm_dt not in (F32,) else F32
            if mm_dt == R32 or fp16w:
                hm_dt = R32
            hprev = constp.tile([P, KC, Bc], state_dt, tag="hprev")
            nc.vector.memset(hprev[:].bitcast(mybir.dt.uint32), 0)
            hprev_mm = constp.tile([P, KC, Bc], hm_dt, tag="hprev_mm")
            nc.vector.memset(hprev_mm[:].bitcast(mybir.dt.uint32), 0)

            with (
                tc.tile_pool(name="p2in", bufs=2) as p2in,
                tc.tile_pool(name="p2r", bufs=2) as p2r,
                tc.tile_pool(name="p2z", bufs=3) as p2z,
                tc.tile_pool(name="p2ps", bufs=4, space=bass.MemorySpace.PSUM) as p2ps,
            ):
                with tc.For_i(0, T, TB) as t0:
                    xwb = p2in.tile([P, JC, Bc, TB], mm_dt, tag="xwb")
                    ab = p2in.tile([P, JC, Bc, TB], F32, tag="ab")
                    cb = p2in.tile([P, JC, Bc, TB], F32, tag="cb")
                    nc.sync.dma_start(
                        xwb[:],
                        xw_s[:, :, :, ds(t0, TB)].rearrange("jc b p t -> p jc b t"),
                    )
                    nc.sync.dma_start(
                        ab[:],
                        a_s[:, :, :, ds(t0, TB)].rearrange("jc b p t -> p jc b t"),
                    )
                    nc.sync.dma_start(
                        cb[:],
                        c_s[:, :, :, ds(t0, TB)].rearrange("jc b p t -> p jc b t"),
                    )
                    # ring[p, jc, b, k] = h after step t0+k (jc doubles as kc)
                    ring = p2r.tile([P, JC, Bc, TB], state_dt, tag="ring")
                    h_mm = hprev_mm
                    for k in range(TB):
                        hold = (
                            hprev[:, :, :] if k == 0 else ring[:, :, :, k - 1]
                        )
                        ps = p2ps.tile([P, JC, Bc], F32, tag="ps")
                        # seed psum with xw via identity matmuls (independent
                        # of h -> runs ahead of the Wh matmuls)
                        for jc in range(JC):
                            nc.tensor.matmul(
                                ps[:, jc, :],
                                eyesb[:],
                                xwb[:, jc, :, k],
                                start=(jc == 0),
                                stop=False,
                            )
                        # e = (1 - a) * h, also independent of the matmul
                        e = p2z.tile([P, JC, Bc], F32, tag="e")
                        nc.vector.tensor_mul(e[:], cb[:, :, :, k], hold)
                        for jc in range(JC):
                            for kc in range(KC):
                                nc.tensor.matmul(
                                    ps[:, jc, :],
                                    whsb[:, kc, jc, :],
                                    h_mm[:, kc, :],
                                    start=False,
                                    stop=(jc == JC - 1 and kc == KC - 1),
                                )
                        z = p2z.tile([P, JC, Bc], F32, tag="z")
                        nc.scalar.activation(z[:], ps[:], AFT.Tanh)
                        f = p2z.tile([P, JC, Bc], F32, tag="f")
                        nc.vector.tensor_mul(f[:], ab[:, :, :, k], z[:])
                        if k < TB - 1:
                            # matmul feed, computed in parallel with the ring
                            # write (not serially after it)
                            hmk = p2z.tile([P, JC, Bc], hm_dt, tag="hmk")
                            nc.vector.tensor_add(hmk[:], e[:], f[:])
                            h_mm = hmk
                        nc.vector.tensor_add(ring[:, :, :, k], e[:], f[:])
                    nc.vector.tensor_copy(hprev[:], ring[:, :, :, TB - 1])
                    nc.vector.tensor_copy(hprev_mm[:], ring[:, :, :, TB - 1])
                    nc.sync.dma_start(
                        out_s[:, :, :, ds(t0, TB)].rearrange("jc b p t -> p jc b t"),
                        ring[:],
                    )

    nc.compile()
    return nc


def _mm_cfg():
    """Recurrence-matmul dtype config, selectable via LIQUID_MM_DT env."""
    import os

    name = os.environ.get("LIQUID_MM_DT", "fp16").lower()
    if name in ("fp16", "fp16w"):
        return mybir.dt.float16, np.float16
    if name == "fp32":
        return F32, np.float32
    return mybir.dt.float32r, np.float32


def _host_prep(x_seq, Wx, Wh, b, tau_w, tau_b, Bc, T, D, H, mm_np=None):
    if mm_np is None:
        mm_np = _mm_cfg()[1]
    """Layout-only host transforms into the device input format."""
    KC, JC, DC = H // P, H // P, D // P
    whT = np.ascontiguousarray(
        Wh.T.reshape(KC, P, JC, P).transpose(1, 0, 2, 3).astype(mm_np)
    )  # [k, kc, jc, m]
    wxT = np.ascontiguousarray(Wx.T.reshape(DC, P, JC, P).transpose(1, 0, 2, 3))
    twT = np.ascontiguousarray(tau_w.T.reshape(DC, P, JC, P).transpose(1, 0, 2, 3))
    bv = np.ascontiguousarray(b.reshape(JC, P).T)
    tbv = np.ascontiguousarray(tau_b.reshape(JC, P).T)
    eye = np.eye(P, dtype=mm_np)

    in_maps = []
    for c in range(N_CORES):
        xc = x_seq[c * Bc : (c + 1) * Bc]  # [Bc, T, D]
        xT = np.ascontiguousarray(
            xc.transpose(2, 0, 1).reshape(DC, P, Bc, T).transpose(1, 0, 2, 3)
        )  # [p, dc, b, t]
        in_maps.append(
            {"xT": xT, "WhT": whT, "WxT": wxT, "TwT": twT, "bv": bv,
             "tbv": tbv, "eye": eye}
        )
    return in_maps


def kernel(x_seq, Wx, Wh, b, tau_w, tau_b):
    x_seq = np.ascontiguousarray(np.asarray(x_seq, dtype=np.float32))
    Wx = np.ascontiguousarray(np.asarray(Wx, dtype=np.float32))
    Wh = np.ascontiguousarray(np.asarray(Wh, dtype=np.float32))
    b = np.ascontiguousarray(np.asarray(b, dtype=np.float32))
    tau_w = np.ascontiguousarray(np.asarray(tau_w, dtype=np.float32))
    tau_b = np.ascontiguousarray(np.asarray(tau_b, dtype=np.float32))

    B, T, D = x_seq.shape
    H = Wx.shape[0]
    Bc = B // N_CORES

    nc = build_program(Bc=Bc, T=T, D=D, H=H)
    in_maps = _host_prep(x_seq, Wx, Wh, b, tau_w, tau_b, Bc, T, D, H)
    res = run_bass_kernel_spmd(nc, in_maps, core_ids=list(range(N_CORES)))

    JC = H // P
    outs = []
    for c in range(N_CORES):
        o = res.results[c]["out_s"]  # [jc, b, p, t]
        outs.append(o.transpose(1, 3, 0, 2).reshape(Bc, T, H))
    return np.ascontiguousarray(np.concatenate(outs, axis=0))


if __name__ == "__main__":
    B, T, D, H = 16, 2048, 256, 512
    rng = np.random.default_rng(0)
    inputs = {
        "x_seq": rng.standard_normal((B, T, D), dtype=np.float32),
        "Wx": (rng.standard_normal((H, D), dtype=np.float32) * 0.1),
        "Wh": (rng.standard_normal((H, H), dtype=np.float32) * 0.1),
        "b": np.zeros((H,), np.float32),
        "tau_w": rng.standard_normal((H, D), dtype=np.float32) / np.sqrt(D),
        "tau_b": np.zeros((H,), np.float32),
    }
    out = kernel(**inputs)
    print(out.shape, out.dtype)
